# revision 10
# baseline (speedup 1.0000x reference)
"""Causal self-attention on 8 Trainium2 NeuronCores.

Sharding: batch (2) x head-groups (4 heads each) -> 8 cores. Each core
computes Q/K/V projections for its 4 heads, causal attention, and the
partial output projection for its head rows of Wo; the host sums the 4
partials per batch.

All matmul operands are bf16 (PSUM accumulation fp32): same PE rate as
float32r (1 cycle/row) but half the DMA/SBUF/DVE traffic. Device layout
is fully transposed: QT/KT [m, s] from W-stationary matmuls, scoresT
[sk, sq] feed an augmented-V matmul whose ones-column produces the
softmax denominator for free, attendedT [m, s] is the stationary for
the output projection.

Attention runs in sq-subrange segments (4x512-wide for sq<1024, 8x
256-wide with merged-sub exp above) so finished sq tiles project+DMA
out early; attended matmuls trail scores by one chunk so the exp
round-trip stays off the PE critical path; output-projection and V
pieces are injected as ready PE filler to keep the HAM clock warm.
"""

from collections import deque
from contextlib import ExitStack

import numpy as np
import ml_dtypes

import concourse.bacc as bacc
import concourse.bass as bass  # noqa: F401
import concourse.mybir as mybir
import concourse.tile as tile
from concourse.bass_utils import run_bass_kernel_spmd

P = 128
B, S, D, H, HD = 2, 2048, 1024, 16, 64
NCORES = 8
HC = 4              # heads per core
MC = HC * HD        # 256 output columns (m) per core
VW = HC * (HD + 1)  # V'' width: 4 heads x (64 vals + 1 ones col)
NDC = D // P        # 8 contraction chunks
NST = S // P        # 16 sequence tiles
F32 = mybir.dt.float32
R32 = mybir.dt.float32r
BF = mybir.dt.bfloat16
ONE_BITS_F32 = 0x3F800000
ONE_BITS_BF = 0x3F80

_NC_CACHE = None


def _build_program():
    nc = bacc.Bacc("TRN2", target_bir_lowering=False, debug=False)
    xt = nc.dram_tensor("xt", [D, S], BF, kind="ExternalInput").ap()
    wq = nc.dram_tensor("wq", [D, MC], BF, kind="ExternalInput").ap()
    wk = nc.dram_tensor("wk", [D, MC], BF, kind="ExternalInput").ap()
    wv = nc.dram_tensor("wv", [D, VW], BF, kind="ExternalInput").ap()
    wo = nc.dram_tensor("wo", [MC, D], BF, kind="ExternalInput").ap()
    tri = nc.dram_tensor("tri", [P, P], BF, kind="ExternalInput").ap()
    out = nc.dram_tensor("out", [S, D], F32, kind="ExternalOutput").ap()

    with tile.TileContext(nc) as tc, ExitStack() as ctx, \
            nc.allow_low_precision(reason="bf16 matmul pipeline"):
        constp = ctx.enter_context(tc.tile_pool(name="constp", bufs=1))
        xtp = ctx.enter_context(tc.tile_pool(name="xtp", bufs=1))
        kxp = ctx.enter_context(tc.tile_pool(name="kxp", bufs=1))
        wp = ctx.enter_context(tc.tile_pool(name="wp", bufs=1))
        qkp = ctx.enter_context(tc.tile_pool(name="qkp", bufs=1))
        vp = ctx.enter_context(tc.tile_pool(name="vp", bufs=1))
        attp = ctx.enter_context(tc.tile_pool(name="attp", bufs=1))
        etp = ctx.enter_context(tc.tile_pool(name="etp", bufs=1))
        drp = ctx.enter_context(tc.tile_pool(name="drp", bufs=1))
        otp = ctx.enter_context(tc.tile_pool(name="otp", bufs=1))
        ps = ctx.enter_context(tc.tile_pool(name="ps", bufs=1, space="PSUM"))

        # constants: causal-keep mask tri[r,c] = (r<=c) in bf16, plus a
        # f32r ones row for the denominator broadcast matmul
        tri_sb = constp.tile([P, P], BF)
        nc.sync.dma_start(tri_sb, tri)
        ones_r = constp.tile([1, 64], BF)
        nc.vector.memset(ones_r.bitcast(mybir.dt.uint16), ONE_BITS_BF)

        wq_sb = wp.tile([P, NDC, MC], BF)
        wk_sb = wp.tile([P, NDC, MC], BF)
        wv_sb = wp.tile([P, NDC, VW], BF)
        wo_sb = wp.tile([P, 2, D], BF)
        xt_sb = xtp.tile([P, NDC, S], BF)
        # DMA priority order: first QK weights + the slab-0 x chunks the
        # first matmuls need, then the rest slab-major so the slab-
        # sequential QK loop consumes pieces as they land.
        for dc in range(NDC):
            nc.sync.dma_start(wq_sb[:, dc, :], wq[dc * P:(dc + 1) * P, :])
            nc.sync.dma_start(wk_sb[:, dc, :], wk[dc * P:(dc + 1) * P, :])
        for slab in range(4):
            s0 = slab * 512
            for dc in range(NDC):
                nc.sync.dma_start(xt_sb[:, dc, s0:s0 + 512],
                                  xt[dc * P:(dc + 1) * P, s0:s0 + 512])
        for dc in range(NDC):
            nc.sync.dma_start(wv_sb[:, dc, :], wv[dc * P:(dc + 1) * P, :])
        for mc2 in range(2):
            nc.sync.dma_start(wo_sb[:, mc2, :], wo[mc2 * P:(mc2 + 1) * P, :])

        # ---- Q/K projections: QT/KT [m, s] (W stationary) ----
        # KT in per-head layout padded to 128 contraction rows (zeros in
        # the other head's rows) so the stationary registers as full-
        # array PE activity for the HAM clock gate.
        qt_sb = qkp.tile([P, 2, S], BF)
        kt_pad = kxp.tile([P, HC, S], BF)
        v_sb = vp.tile([P, NST, VW], BF)
        att_sb = attp.tile([P, 2, S], BF)
        for hh in range(HC):
            zo = 64 - (hh % 2) * 64
            nc.vector.memset(
                kt_pad[zo:zo + 64, hh, :].bitcast(mybir.dt.uint16), 0)

        # slab-sequential: one 512-col slab unit at a time (Q + K psum
        # tiles, 1 bank each) so only ~2 units are ever in flight and
        # the first matmul fires as soon as slab-0/dc-0 pieces land
        drain_flip = [0]

        def emit_qk_unit(slab, mc2):
            s0 = slab * 512
            pq = ps.tile([P, 512], F32, tag="ps", bufs=3, name="pq")
            pk = ps.tile([P, 512], F32, tag="ps", bufs=3, name="pk")
            for dc in range(NDC):
                nc.tensor.matmul(pq[:, :],
                                 wq_sb[:, dc, mc2 * P:(mc2 + 1) * P],
                                 xt_sb[:, dc, s0:s0 + 512],
                                 start=(dc == 0), stop=(dc == NDC - 1))
                nc.tensor.matmul(pk[:, :],
                                 wk_sb[:, dc, mc2 * P:(mc2 + 1) * P],
                                 xt_sb[:, dc, s0:s0 + 512],
                                 start=(dc == 0), stop=(dc == NDC - 1))
            if drain_flip[0] % 2:
                nc.scalar.copy(qt_sb[:, mc2, s0:s0 + 512], pq[:, :])
            else:
                nc.vector.tensor_copy(qt_sb[:, mc2, s0:s0 + 512], pq[:, :])
            drain_flip[0] += 1
            nc.vector.tensor_copy(kt_pad[0:64, 2 * mc2, s0:s0 + 512],
                                  pk[0:64, :])
            nc.vector.tensor_copy(kt_pad[64:128, 2 * mc2 + 1, s0:s0 + 512],
                                  pk[64:128, :])

        for slab in range(4):
            for mc2 in range(2):
                emit_qk_unit(slab, mc2)

        # ---- V projection (st tiles 0..7 now, 8..15 as filler) ----
        def emit_vproj(st):
            pv = ps.tile([P, VW], F32, tag="pv", bufs=1, name="pv")
            for dc in range(NDC):
                nc.tensor.matmul(pv[:, :],
                                 xt_sb[:, dc, st * P:(st + 1) * P],
                                 wv_sb[:, dc, :],
                                 start=(dc == 0), stop=(dc == NDC - 1))
            nc.vector.tensor_copy(v_sb[:, st, :], pv[:, :])
            for j in range(HC):
                nc.vector.memset(
                    v_sb[:, st:st + 1,
                         j * (HD + 1) + HD].bitcast(mybir.dt.uint16),
                    ONE_BITS_BF)

        for st in range(8):
            emit_vproj(st)

        # ---- output projection: one 512-d-col piece per psum tile ----
        op_flip = [0]

        def emit_op_piece(st, a):
            po = ps.tile([P, 512], F32, tag="po", bufs=1, name="po")
            for mc2 in (0, 1):
                nc.tensor.matmul(po[:, :],
                                 att_sb[:, mc2, st * P:(st + 1) * P],
                                 wo_sb[:, mc2, a:a + 512],
                                 start=(mc2 == 0), stop=(mc2 == 1))
            ot = otp.tile([P, 512], F32, tag="ot", bufs=3, name="ot")
            if op_flip[0] % 2:
                nc.scalar.copy(ot[:, :], po[:, :])
            else:
                nc.vector.tensor_copy(ot[:, :], po[:, :])
            op_flip[0] += 1
            nc.sync.dma_start(out[st * P:(st + 1) * P, a:a + 512], ot[:, :])

        def emit_op(st):
            emit_op_piece(st, 0)
            emit_op_piece(st, 512)

        # ---- attention segments ----
        # pending normalize-broadcasts, deferred into the next segment
        # so the PE absorbs them between attention matmuls
        pending = []

        def flush_pending():
            while pending:
                mcq, q0, W, dra, drb = pending.pop(0)
                pb = ps.tile([P, 512], F32, tag="pb", bufs=1, name="pb")
                nc.tensor.matmul(pb[0:64, 0:W], ones_r, dra[:, 0:W],
                                 start=True, stop=True)
                nc.tensor.matmul(pb[64:128, 0:W], ones_r, drb[:, 0:W],
                                 start=True, stop=True, tile_position=(0, 64))
                nc.vector.tensor_mul(att_sb[0:64, mcq, q0:q0 + W],
                                     att_sb[0:64, mcq, q0:q0 + W],
                                     pb[0:64, 0:W])
                nc.vector.tensor_mul(att_sb[64:128, mcq, q0:q0 + W],
                                     att_sb[64:128, mcq, q0:q0 + W],
                                     pb[64:128, 0:W])

        def segment(mcq, q0, W, merged, fillers):
            nch = (q0 + W) // P  # chunks 0..nch-1 (sk < q0+W)
            pas = [ps.tile([65, W], F32, tag="pa", bufs=2, name=f"pa{s_}")
                   for s_ in range(2)]
            att_q = deque()
            fillers = list(fillers)

            def emit_att(item):
                i, et, c0 = item
                for sub in range(2):
                    hh = 2 * mcq + sub
                    vlo = hh * (HD + 1)
                    nc.tensor.matmul(
                        pas[sub][0:HD + 1, c0:W],
                        v_sb[:, i, vlo:vlo + HD + 1],
                        et[:, sub, c0:W] if merged else et[sub][:, c0:W],
                        start=(i == 0), stop=(i == nch - 1))

            for i in range(nch):
                c0 = max(0, i * P - q0)
                dw = min(P, W - c0) if i * P >= q0 else 0
                if merged:
                    pscr = ps.tile([P, 2, W], F32, tag="ps", bufs=3, name="pscr")
                    et = etp.tile([P, 2, W], BF, tag="et", bufs=6)
                    for sub in range(2):
                        hh = 2 * mcq + sub
                        nc.tensor.matmul(
                            pscr[:, sub, c0:W],
                            kt_pad[:, hh, i * P:(i + 1) * P],
                            qt_sb[:, mcq, q0 + c0:q0 + W],
                            start=True, stop=True)
                    nc.scalar.activation(
                        out=et[:, :, c0:W], in_=pscr[:, :, c0:W],
                        func=mybir.ActivationFunctionType.Exp, scale=0.125)
                    if dw:
                        for sub in range(2):
                            nc.gpsimd.tensor_mul(et[:, sub, c0:c0 + dw],
                                                 et[:, sub, c0:c0 + dw],
                                                 tri_sb[:, 0:dw])
                    att_q.append((i, et, c0))
                else:
                    ets = []
                    for sub in range(2):
                        hh = 2 * mcq + sub
                        pscr = ps.tile([P, W], F32, tag="ps", bufs=3, name="pscr")
                        et1 = etp.tile([P, W], BF, tag="et", bufs=6,
                                       name="et1")
                        nc.tensor.matmul(
                            pscr[:, c0:W],
                            kt_pad[:, hh, i * P:(i + 1) * P],
                            qt_sb[:, mcq, q0 + c0:q0 + W],
                            start=True, stop=True)
                        nc.scalar.activation(
                            out=et1[:, c0:W], in_=pscr[:, c0:W],
                            func=mybir.ActivationFunctionType.Exp,
                            scale=0.125)
                        if dw:
                            nc.gpsimd.tensor_mul(et1[:, c0:c0 + dw],
                                                 et1[:, c0:c0 + dw],
                                                 tri_sb[:, 0:dw])
                        ets.append(et1)
                    att_q.append((i, ets, c0))
                if len(att_q) >= 2:
                    emit_att(att_q.popleft())
                if i == 2:
                    flush_pending()
                if fillers and i >= 3 and i % 2 == 1:
                    fillers.pop(0)()
            while att_q:
                emit_att(att_q.popleft())
            for f in fillers:
                f()

            # normalize: row HD of pa holds the softmax denominator.
            # Copy the attended rows out (bf16) on gpsimd, reciprocal
            # straight off the psum row on DVE; the recip broadcast is
            # deferred into the next segment (flush_pending).
            dra = drp.tile([1, W], BF, tag="dra", bufs=2, name="dra")
            drb = drp.tile([1, W], BF, tag="drb", bufs=2, name="drb")
            for sub, dr in ((0, dra), (1, drb)):
                drs = drp.tile([1, W], F32, tag="drs", bufs=2, name="drs")
                if sub == 0:
                    nc.scalar.copy(att_sb[0:64, mcq, q0:q0 + W],
                                   pas[0][0:64, 0:W])
                else:
                    nc.vector.tensor_copy(att_sb[64:128, mcq, q0:q0 + W],
                                          pas[1][0:64, 0:W])
                nc.vector.tensor_copy(drs[:, 0:W],
                                      pas[sub][HD:HD + 1, 0:W])
                nc.vector.reciprocal_approx_fast(
                    out=drs[:, 0:W], in_=drs[:, 0:W])
                nc.vector.tensor_copy(dr[:, 0:W], drs[:, 0:W])
            pending.append((mcq, q0, W, dra, drb))

        segs = [
            (0, 0, 512, False, [lambda: emit_vproj(8), lambda: emit_vproj(9)]),
            (1, 0, 512, False, [lambda: emit_vproj(10),
                                lambda: emit_vproj(11)]),
            (0, 512, 512, False, [lambda: emit_op(0), lambda: emit_op(1)]),
            (1, 512, 512, False, [lambda: emit_op(2), lambda: emit_op(3)]),
            (1, 1024, 256, True, [lambda: emit_op(4),
                                  lambda: emit_vproj(12)]),
            (0, 1024, 256, True, [lambda: emit_op(5),
                                  lambda: emit_vproj(13)]),
            (1, 1280, 256, True, [lambda: emit_op(6),
                                  lambda: emit_vproj(14)]),
            (0, 1280, 256, True, [lambda: emit_op(7),
                                  lambda: emit_vproj(15)]),
            (1, 1536, 256, True, [lambda: emit_op(8), lambda: emit_op(9)]),
            (0, 1536, 256, True, [lambda: emit_op(10), lambda: emit_op(11)]),
            (1, 1792, 256, True, [lambda: emit_op(12)]),
            (0, 1792, 256, True, [lambda: emit_op(13)]),
        ]
        for mcq, q0, W, merged, fillers in segs:
            segment(mcq, q0, W, merged, fillers)
        flush_pending()
        emit_op(14)
        emit_op(15)

    nc.compile()
    return nc


def get_program():
    global _NC_CACHE
    if _NC_CACHE is None:
        _NC_CACHE = _build_program()
    return _NC_CACHE


def prepare_in_maps(inputs):
    BF_NP = ml_dtypes.bfloat16
    x = np.asarray(inputs["x"], dtype=np.float32)
    Wq = np.asarray(inputs["Wq"], dtype=np.float32)
    Wk = np.asarray(inputs["Wk"], dtype=np.float32)
    Wv = np.asarray(inputs["Wv"], dtype=np.float32)
    Wo = np.asarray(inputs["Wo"], dtype=np.float32)
    xts = [np.ascontiguousarray(x[b].T).astype(BF_NP) for b in range(B)]
    tri = np.triu(np.ones((P, P), dtype=np.float32)).astype(BF_NP)
    in_maps = []
    for c in range(NCORES):
        b = c // 4
        hg = c % 4
        cols = slice(hg * MC, (hg + 1) * MC)
        wv_c = np.zeros((D, VW), np.float32)
        for j in range(HC):
            wv_c[:, j * (HD + 1):j * (HD + 1) + HD] = \
                Wv[:, hg * MC + j * HD:hg * MC + (j + 1) * HD]
        in_maps.append({
            "xt": xts[b],
            "wq": np.ascontiguousarray(Wq[:, cols]).astype(BF_NP),
            "wk": np.ascontiguousarray(Wk[:, cols]).astype(BF_NP),
            "wv": wv_c.astype(BF_NP),
            "wo": np.ascontiguousarray(Wo[cols, :]).astype(BF_NP),
            "tri": tri,
        })
    return in_maps


def gather_output(results):
    outs = [np.asarray(results[c]["out"], dtype=np.float32)
            for c in range(NCORES)]
    return np.stack([outs[0] + outs[1] + outs[2] + outs[3],
                     outs[4] + outs[5] + outs[6] + outs[7]])


def kernel(**inputs) -> np.ndarray:
    nc = get_program()
    in_maps = prepare_in_maps(inputs)
    res = run_bass_kernel_spmd(nc, in_maps, list(range(NCORES)))
    return gather_output(res.results)


# revision 11
# speedup vs baseline: 1.0550x; 1.0550x over previous
"""Causal self-attention on 8 Trainium2 NeuronCores.

Sharding: batch (2) x head-groups (4 heads each) -> 8 cores. Each core
computes Q/K/V projections for its 4 heads, causal attention, and the
partial output projection for its head rows of Wo; the host sums the 4
partials per batch.

All matmul operands are bf16 (PSUM accumulation fp32): same PE rate as
float32r (1 cycle/row) but half the DMA/SBUF/DVE traffic. Device layout
is fully transposed: QT/KT [m, s] from W-stationary matmuls, scoresT
[sk, sq] feed an augmented-V matmul whose ones-column produces the
softmax denominator for free, attendedT [m, s] is the stationary for
the output projection.

Attention runs in sq-subrange segments (4x512-wide for sq<1024, 8x
256-wide with merged-sub exp above) so finished sq tiles project+DMA
out early; attended matmuls trail scores by one chunk so the exp
round-trip stays off the PE critical path; output-projection and V
pieces are injected as ready PE filler to keep the HAM clock warm.
"""

from collections import deque
from contextlib import ExitStack

import numpy as np
import ml_dtypes

import concourse.bacc as bacc
import concourse.bass as bass  # noqa: F401
import concourse.mybir as mybir
import concourse.tile as tile
from concourse.bass_utils import run_bass_kernel_spmd

P = 128
B, S, D, H, HD = 2, 2048, 1024, 16, 64
NCORES = 8
HC = 4              # heads per core
MC = HC * HD        # 256 output columns (m) per core
VW = HC * (HD + 1)  # V'' width: 4 heads x (64 vals + 1 ones col)
NDC = D // P        # 8 contraction chunks
NST = S // P        # 16 sequence tiles
F32 = mybir.dt.float32
R32 = mybir.dt.float32r
BF = mybir.dt.bfloat16
ONE_BITS_F32 = 0x3F800000
ONE_BITS_BF = 0x3F80

_NC_CACHE = None


def _build_program():
    nc = bacc.Bacc("TRN2", target_bir_lowering=False, debug=False)
    xt = nc.dram_tensor("xt", [D, S], BF, kind="ExternalInput").ap()
    wq = nc.dram_tensor("wq", [D, MC], BF, kind="ExternalInput").ap()
    wk = nc.dram_tensor("wk", [D, MC], BF, kind="ExternalInput").ap()
    wv = nc.dram_tensor("wv", [D, VW], BF, kind="ExternalInput").ap()
    wo = nc.dram_tensor("wo", [MC, D], BF, kind="ExternalInput").ap()
    tri = nc.dram_tensor("tri", [P, P], BF, kind="ExternalInput").ap()
    out = nc.dram_tensor("out", [S, D], F32, kind="ExternalOutput").ap()

    with tile.TileContext(nc) as tc, ExitStack() as ctx, \
            nc.allow_low_precision(reason="bf16 matmul pipeline"):
        constp = ctx.enter_context(tc.tile_pool(name="constp", bufs=1))
        xtp = ctx.enter_context(tc.tile_pool(name="xtp", bufs=1))
        kxp = ctx.enter_context(tc.tile_pool(name="kxp", bufs=1))
        wp = ctx.enter_context(tc.tile_pool(name="wp", bufs=1))
        qkp = ctx.enter_context(tc.tile_pool(name="qkp", bufs=1))
        vp = ctx.enter_context(tc.tile_pool(name="vp", bufs=1))
        attp = ctx.enter_context(tc.tile_pool(name="attp", bufs=1))
        etp = ctx.enter_context(tc.tile_pool(name="etp", bufs=1))
        drp = ctx.enter_context(tc.tile_pool(name="drp", bufs=1))
        otp = ctx.enter_context(tc.tile_pool(name="otp", bufs=1))
        ps = ctx.enter_context(tc.tile_pool(name="ps", bufs=1, space="PSUM"))

        # constants: causal-keep mask tri[r,c] = (r<=c) in bf16, plus a
        # f32r ones row for the denominator broadcast matmul
        tri_sb = constp.tile([P, P], BF)
        ones_r = constp.tile([1, 64], BF)
        nc.vector.memset(ones_r.bitcast(mybir.dt.uint16), ONE_BITS_BF)

        wq_sb = wp.tile([P, NDC, MC], BF)
        wk_sb = wp.tile([P, NDC, MC], BF)
        wv_sb = wp.tile([P, NDC, VW], BF)
        wo_sb = wp.tile([P, 2, D], BF)
        xt_sb = xtp.tile([P, NDC, S], BF)
        # Batched dc-pair DMAs spread over the three trigger queues
        # (sync + scalar HW DGE, gpsimd SWDGE) in consumption order:
        # slab-0 QK inputs first, then the remaining x slabs, V/O weights.
        xt_r = xt.rearrange("(dc p) s -> p dc s", p=P)
        wq_r = wq.rearrange("(dc p) m -> p dc m", p=P)
        wk_r = wk.rearrange("(dc p) m -> p dc m", p=P)
        wv_r = wv.rearrange("(dc p) m -> p dc m", p=P)
        wo_r = wo.rearrange("(m p) d -> p m d", p=P)
        for j in range(4):
            dc2 = slice(2 * j, 2 * j + 2)
            nc.gpsimd.dma_start(wq_sb[:, dc2, :], wq_r[:, dc2, :])
            nc.scalar.dma_start(wk_sb[:, dc2, :], wk_r[:, dc2, :])
            nc.sync.dma_start(xt_sb[:, dc2, 0:512], xt_r[:, dc2, 0:512])
        for j in range(4):
            dc2 = slice(2 * j, 2 * j + 2)
            nc.sync.dma_start(xt_sb[:, dc2, 512:1024],
                              xt_r[:, dc2, 512:1024])
            nc.scalar.dma_start(xt_sb[:, dc2, 1024:1536],
                                xt_r[:, dc2, 1024:1536])
            nc.gpsimd.dma_start(xt_sb[:, dc2, 1536:2048],
                                xt_r[:, dc2, 1536:2048])
        for j in range(4):
            dc2 = slice(2 * j, 2 * j + 2)
            nc.scalar.dma_start(wv_sb[:, dc2, :], wv_r[:, dc2, :])
        nc.gpsimd.dma_start(wo_sb[:, :, :], wo_r)
        nc.sync.dma_start(tri_sb, tri)

        # ---- Q/K projections: QT/KT [m, s] (W stationary) ----
        # KT in per-head layout padded to 128 contraction rows (zeros in
        # the other head's rows) so the stationary registers as full-
        # array PE activity for the HAM clock gate.
        qt_sb = qkp.tile([P, 2, S], BF)
        kt_pad = kxp.tile([P, HC, S], BF)
        v_sb = vp.tile([P, NST, VW], BF)
        att_sb = attp.tile([P, 2, S], BF)
        for j in range(HC):
            nc.gpsimd.memset(
                v_sb[:, 0:NST, j * (HD + 1) + HD].bitcast(mybir.dt.uint16),
                ONE_BITS_BF)
        for hh in range(HC):
            zo = 64 - (hh % 2) * 64
            eng = nc.vector if hh % 2 else nc.gpsimd
            eng.memset(
                kt_pad[zo:zo + 64, hh, :].bitcast(mybir.dt.uint16), 0)

        # slab-sequential: one 512-col slab unit at a time (Q + K psum
        # tiles, 1 bank each) so only ~2 units are ever in flight and
        # the first matmul fires as soon as slab-0/dc-0 pieces land
        drain_flip = [0]

        def emit_qk_unit(slab, mc2):
            s0 = slab * 512
            pq = ps.tile([P, 512], F32, tag="ps", bufs=3, name="pq")
            pk = ps.tile([P, 512], F32, tag="ps", bufs=3, name="pk")
            for dc in range(NDC):
                nc.tensor.matmul(pq[:, :],
                                 wq_sb[:, dc, mc2 * P:(mc2 + 1) * P],
                                 xt_sb[:, dc, s0:s0 + 512],
                                 start=(dc == 0), stop=(dc == NDC - 1))
                nc.tensor.matmul(pk[:, :],
                                 wk_sb[:, dc, mc2 * P:(mc2 + 1) * P],
                                 xt_sb[:, dc, s0:s0 + 512],
                                 start=(dc == 0), stop=(dc == NDC - 1))
            if drain_flip[0] % 2:
                nc.scalar.copy(qt_sb[:, mc2, s0:s0 + 512], pq[:, :])
            else:
                nc.vector.tensor_copy(qt_sb[:, mc2, s0:s0 + 512], pq[:, :])
            drain_flip[0] += 1
            nc.vector.tensor_copy(kt_pad[0:64, 2 * mc2, s0:s0 + 512],
                                  pk[0:64, :])
            nc.vector.tensor_copy(kt_pad[64:128, 2 * mc2 + 1, s0:s0 + 512],
                                  pk[64:128, :])

        for slab in range(4):
            for mc2 in range(2):
                emit_qk_unit(slab, mc2)

        # ---- V projection (st tiles 0..7 now, 8..15 as filler) ----
        def emit_vproj(st):
            pv = ps.tile([P, VW], F32, tag="pv", bufs=1, name="pv")
            for dc in range(NDC):
                nc.tensor.matmul(pv[:, :],
                                 xt_sb[:, dc, st * P:(st + 1) * P],
                                 wv_sb[:, dc, :],
                                 start=(dc == 0), stop=(dc == NDC - 1))
            pv4 = pv.rearrange("p (j c) -> p j c", j=HC)
            v4 = v_sb[:, st, :].rearrange("p (j c) -> p j c", j=HC)
            nc.vector.tensor_copy(v4[:, :, 0:HD], pv4[:, :, 0:HD])

        for st in range(8):
            emit_vproj(st)

        # ---- output projection: one 512-d-col piece per psum tile ----
        op_flip = [0]

        def emit_op_piece(st, a):
            po = ps.tile([P, 512], F32, tag="po", bufs=1, name="po")
            for mc2 in (0, 1):
                nc.tensor.matmul(po[:, :],
                                 att_sb[:, mc2, st * P:(st + 1) * P],
                                 wo_sb[:, mc2, a:a + 512],
                                 start=(mc2 == 0), stop=(mc2 == 1))
            ot = otp.tile([P, 512], F32, tag="ot", bufs=3, name="ot")
            nc.vector.tensor_copy(ot[:, :], po[:, :])
            op_flip[0] += 1
            nc.sync.dma_start(out[st * P:(st + 1) * P, a:a + 512], ot[:, :])

        def emit_op(st):
            emit_op_piece(st, 0)
            emit_op_piece(st, 512)

        # ---- attention segments ----
        # pending normalize-broadcasts, deferred into the next segment
        # so the PE absorbs them between attention matmuls
        pending = []

        def flush_pending():
            while pending:
                mcq, q0, W, dra, drb = pending.pop(0)
                pb = ps.tile([P, 512], F32, tag="pb", bufs=1, name="pb")
                nc.tensor.matmul(pb[0:64, 0:W], ones_r, dra[:, 0:W],
                                 start=True, stop=True)
                nc.tensor.matmul(pb[64:128, 0:W], ones_r, drb[:, 0:W],
                                 start=True, stop=True, tile_position=(0, 64))
                nc.vector.tensor_mul(att_sb[0:64, mcq, q0:q0 + W],
                                     att_sb[0:64, mcq, q0:q0 + W],
                                     pb[0:64, 0:W])
                nc.vector.tensor_mul(att_sb[64:128, mcq, q0:q0 + W],
                                     att_sb[64:128, mcq, q0:q0 + W],
                                     pb[64:128, 0:W])

        def segment(mcq, q0, W, merged, fillers):
            nch = (q0 + W) // P  # chunks 0..nch-1 (sk < q0+W)
            pas = [ps.tile([65, W], F32, tag="pa", bufs=2, name=f"pa{s_}")
                   for s_ in range(2)]
            att_q = deque()
            fillers = list(fillers)

            def emit_att(item):
                i, et, c0 = item
                for sub in range(2):
                    hh = 2 * mcq + sub
                    vlo = hh * (HD + 1)
                    nc.tensor.matmul(
                        pas[sub][0:HD + 1, c0:W],
                        v_sb[:, i, vlo:vlo + HD + 1],
                        et[:, sub, c0:W] if merged else et[sub][:, c0:W],
                        start=(i == 0), stop=(i == nch - 1))

            for i in range(nch):
                c0 = max(0, i * P - q0)
                dw = min(P, W - c0) if i * P >= q0 else 0
                if merged:
                    pscr = ps.tile([P, 2, W], F32, tag="ps", bufs=3, name="pscr")
                    et = etp.tile([P, 2, W], BF, tag="et", bufs=6)
                    for sub in range(2):
                        hh = 2 * mcq + sub
                        nc.tensor.matmul(
                            pscr[:, sub, c0:W],
                            kt_pad[:, hh, i * P:(i + 1) * P],
                            qt_sb[:, mcq, q0 + c0:q0 + W],
                            start=True, stop=True)
                    nc.scalar.activation(
                        out=et[:, :, c0:W], in_=pscr[:, :, c0:W],
                        func=mybir.ActivationFunctionType.Exp, scale=0.125)
                    if dw:
                        for sub in range(2):
                            nc.gpsimd.tensor_mul(et[:, sub, c0:c0 + dw],
                                                 et[:, sub, c0:c0 + dw],
                                                 tri_sb[:, 0:dw])
                    att_q.append((i, et, c0))
                else:
                    ets = []
                    for sub in range(2):
                        hh = 2 * mcq + sub
                        pscr = ps.tile([P, W], F32, tag="ps", bufs=3, name="pscr")
                        et1 = etp.tile([P, W], BF, tag="et", bufs=6,
                                       name="et1")
                        nc.tensor.matmul(
                            pscr[:, c0:W],
                            kt_pad[:, hh, i * P:(i + 1) * P],
                            qt_sb[:, mcq, q0 + c0:q0 + W],
                            start=True, stop=True)
                        nc.scalar.activation(
                            out=et1[:, c0:W], in_=pscr[:, c0:W],
                            func=mybir.ActivationFunctionType.Exp,
                            scale=0.125)
                        if dw:
                            nc.gpsimd.tensor_mul(et1[:, c0:c0 + dw],
                                                 et1[:, c0:c0 + dw],
                                                 tri_sb[:, 0:dw])
                        ets.append(et1)
                    att_q.append((i, ets, c0))
                if len(att_q) >= 2:
                    emit_att(att_q.popleft())
                if i == 2:
                    flush_pending()
                if fillers and i >= 3 and i % 2 == 1:
                    fillers.pop(0)()
            while att_q:
                emit_att(att_q.popleft())
            for f in fillers:
                f()

            # normalize: row HD of pa holds the softmax denominator.
            # Copy the attended rows out (bf16) on gpsimd, reciprocal
            # straight off the psum row on DVE; the recip broadcast is
            # deferred into the next segment (flush_pending).
            dra = drp.tile([1, W], BF, tag="dra", bufs=2, name="dra")
            drb = drp.tile([1, W], BF, tag="drb", bufs=2, name="drb")
            for sub, dr in ((0, dra), (1, drb)):
                drs = drp.tile([1, W], F32, tag="drs", bufs=2, name="drs")
                nc.vector.tensor_copy(
                    att_sb[sub * 64:sub * 64 + 64, mcq, q0:q0 + W],
                    pas[sub][0:64, 0:W])
                nc.vector.tensor_copy(drs[:, 0:W],
                                      pas[sub][HD:HD + 1, 0:W])
                nc.vector.reciprocal_approx_fast(
                    out=drs[:, 0:W], in_=drs[:, 0:W])
                nc.vector.tensor_copy(dr[:, 0:W], drs[:, 0:W])
            pending.append((mcq, q0, W, dra, drb))

        segs = [
            (0, 0, 512, False, [lambda: emit_vproj(8), lambda: emit_vproj(9)]),
            (1, 0, 512, False, [lambda: emit_vproj(10),
                                lambda: emit_vproj(11)]),
            (0, 512, 512, False, [lambda: emit_op(0), lambda: emit_op(1)]),
            (1, 512, 512, False, [lambda: emit_op(2), lambda: emit_op(3)]),
            (1, 1024, 256, True, [lambda: emit_op(4),
                                  lambda: emit_vproj(12)]),
            (0, 1024, 256, True, [lambda: emit_op(5),
                                  lambda: emit_vproj(13)]),
            (1, 1280, 256, True, [lambda: emit_op(6),
                                  lambda: emit_vproj(14)]),
            (0, 1280, 256, True, [lambda: emit_op(7),
                                  lambda: emit_vproj(15)]),
            (1, 1536, 256, True, [lambda: emit_op(8), lambda: emit_op(9)]),
            (0, 1536, 256, True, [lambda: emit_op(10), lambda: emit_op(11)]),
            (1, 1792, 256, True, [lambda: emit_op(12)]),
            (0, 1792, 256, True, [lambda: emit_op(13)]),
        ]
        for mcq, q0, W, merged, fillers in segs:
            segment(mcq, q0, W, merged, fillers)
        flush_pending()
        emit_op(14)
        emit_op(15)

    nc.compile()
    return nc


def get_program():
    global _NC_CACHE
    if _NC_CACHE is None:
        _NC_CACHE = _build_program()
    return _NC_CACHE


def prepare_in_maps(inputs):
    BF_NP = ml_dtypes.bfloat16
    x = np.asarray(inputs["x"], dtype=np.float32)
    Wq = np.asarray(inputs["Wq"], dtype=np.float32)
    Wk = np.asarray(inputs["Wk"], dtype=np.float32)
    Wv = np.asarray(inputs["Wv"], dtype=np.float32)
    Wo = np.asarray(inputs["Wo"], dtype=np.float32)
    xts = [np.ascontiguousarray(x[b].T).astype(BF_NP) for b in range(B)]
    tri = np.triu(np.ones((P, P), dtype=np.float32)).astype(BF_NP)
    in_maps = []
    for c in range(NCORES):
        b = c // 4
        hg = c % 4
        cols = slice(hg * MC, (hg + 1) * MC)
        wv_c = np.zeros((D, VW), np.float32)
        for j in range(HC):
            wv_c[:, j * (HD + 1):j * (HD + 1) + HD] = \
                Wv[:, hg * MC + j * HD:hg * MC + (j + 1) * HD]
        in_maps.append({
            "xt": xts[b],
            "wq": np.ascontiguousarray(Wq[:, cols]).astype(BF_NP),
            "wk": np.ascontiguousarray(Wk[:, cols]).astype(BF_NP),
            "wv": wv_c.astype(BF_NP),
            "wo": np.ascontiguousarray(Wo[cols, :]).astype(BF_NP),
            "tri": tri,
        })
    return in_maps


def gather_output(results):
    outs = [np.asarray(results[c]["out"], dtype=np.float32)
            for c in range(NCORES)]
    return np.stack([outs[0] + outs[1] + outs[2] + outs[3],
                     outs[4] + outs[5] + outs[6] + outs[7]])


def kernel(**inputs) -> np.ndarray:
    nc = get_program()
    in_maps = prepare_in_maps(inputs)
    res = run_bass_kernel_spmd(nc, in_maps, list(range(NCORES)))
    return gather_output(res.results)
